# revision 1
# baseline (speedup 1.0000x reference)
"""GCN (MLP pre-encoder + 2 GCNConv layers) on 8 Trainium2 NeuronCores.

Strategy (graph/data parallel, per sharding hint):
- Nodes sharded by rows across 8 cores (12500 -> padded 12544 each).
- Edges partitioned by destination core, sorted by (dest tile, source
  segment); self-loops appended as ordinary edges.
- Symmetric normalization folded into the gathered feature tables:
  g' = dis * g, so each message is an unweighted row gather and the
  per-dest scale dis[d] is applied once per output row.
- Aggregation per 128-dest tile: dma_gather rows of the AllGathered
  table, scatter-add within the tile via a one-hot matmul on the PE
  (onehot[m, d] = (dest_local[m] == d)), accumulated in PSUM.
"""
import os
import sys
sys.path.insert(0, "/opt/trn_rl_repo")
import numpy as np
import concourse.bass as bass
import concourse.bacc as bacc
import concourse.mybir as mybir
from concourse import tile
from concourse.bass_utils import run_bass_kernel_spmd
from contextlib import ExitStack

N_NODES = 100000
N_FEAT = 512
H_MLP = 256
H_GCN = 256
N_CLS = 40
NCORES = 8
RPC = 12500         # real rows per core
RPAD = 12544        # padded rows per core (98 * 128)
NTILE = 98          # dest tiles per core
NPAD = NCORES * RPAD  # padded table rows = 100352
NSEG = 4
SEGR = NPAD // NSEG   # 25088 rows per source segment (< 2**15)
MM_DT = mybir.dt.bfloat16   # dtype for gather tables / onehot / matmul inputs
MM_NP = np.float32 if MM_DT == mybir.dt.float32 else None  # set below
import ml_dtypes
MM_NP = np.float32 if MM_DT == mybir.dt.float32 else ml_dtypes.bfloat16
MM_SZ = 4 if MM_DT == mybir.dt.float32 else 2
K_PHASES = int(os.environ.get("K_PHASES", "5"))  # debug: stop after N phases
C_PAD = 128 if MM_SZ == 2 else 64   # N_CLS padded so gather rows are 256B


def _preprocess(edge_index):
    """Host-side graph preprocessing -> per-core edge structures."""
    row = np.asarray(edge_index[0], dtype=np.int64)
    col = np.asarray(edge_index[1], dtype=np.int64)
    deg = np.bincount(col, minlength=N_NODES).astype(np.float32) + 1.0
    dis = (1.0 / np.sqrt(deg)).astype(np.float32)

    allr = np.arange(N_NODES, dtype=np.int64)
    dest = np.concatenate([row, allr])
    src = np.concatenate([col, allr])

    core = dest // RPC
    dlocal = dest % RPC
    tile_id = dlocal // 128
    dl = (dlocal % 128).astype(np.float32)
    srcp = (src // RPC) * RPAD + (src % RPC)
    seg = srcp // SEGR
    sloc = (srcp % SEGR).astype(np.int16)

    ngrp = NTILE * NSEG
    key = (core * ngrp + tile_id * NSEG + seg).astype(np.int64)
    order = np.argsort(key, kind="stable")
    sk = key[order]
    cnt = np.bincount(key, minlength=NCORES * ngrp).reshape(NCORES, NTILE, NSEG)
    G = cnt.max(axis=0)
    G = ((G + 127) // 128) * 128          # [NTILE, NSEG] padded group sizes
    Goff = np.concatenate([[0], np.cumsum(G.reshape(-1))])[:-1]  # group offsets
    total = int(G.sum())

    starts = np.searchsorted(sk, np.arange(NCORES * ngrp))
    rank = np.arange(len(sk)) - starts[sk]
    ckey = sk // ngrp
    gkey = sk % ngrp
    pos = Goff[gkey] + rank

    idx_arr = np.zeros((NCORES, total), np.int16)          # pad -> idx 0
    dl_arr = np.full((NCORES, total), 255.0, np.float32)   # pad -> no dest
    idx_arr[ckey, pos] = sloc[order]
    dl_arr[ckey, pos] = dl[order]

    gidx = [np.tile(idx_arr[c].reshape(-1, 16).T, (8, 1)).copy() for c in range(NCORES)]
    dloc = [np.ascontiguousarray(dl_arr[c].reshape(-1, 128).T) for c in range(NCORES)]

    dis_pad = np.ones(NCORES * RPAD, np.float32)
    for c in range(NCORES):
        dis_pad[c * RPAD:c * RPAD + RPC] = dis[c * RPC:(c + 1) * RPC]
    diso = [np.ascontiguousarray(
        dis_pad[c * RPAD:(c + 1) * RPAD].reshape(NTILE, 128).T) for c in range(NCORES)]

    return G, Goff, total, gidx, dloc, diso


def _build_program(G, total, b1_nonzero=False):
    """Build the SPMD Bass program (identical across cores)."""
    nch = (G.sum(axis=1) // 128).astype(np.int64)      # chunks per tile
    gs16 = (G // 16).astype(np.int64)                  # idx cols per group
    gs128 = (G // 128).astype(np.int64)                # chunks per group
    total16 = total // 16
    nch_tot = total // 128

    f32 = mybir.dt.float32
    nc = bacc.Bacc("TRN2", target_bir_lowering=False, debug=False,
                   num_devices=NCORES)

    # inputs
    t_xT = nc.dram_tensor("xT", [N_FEAT, RPAD], f32, kind="ExternalInput")
    t_wmlp = nc.dram_tensor("wmlp", [N_FEAT, H_MLP], f32, kind="ExternalInput")
    t_w1 = nc.dram_tensor("w1", [H_MLP, H_GCN], f32, kind="ExternalInput")
    t_w2 = nc.dram_tensor("w2", [H_GCN, C_PAD], f32, kind="ExternalInput")
    t_bmlp = nc.dram_tensor("bmlp", [128, 2], f32, kind="ExternalInput")
    t_b1 = nc.dram_tensor("b1bc", [128, H_GCN], f32, kind="ExternalInput")
    t_b2 = nc.dram_tensor("b2bc", [128, C_PAD], f32, kind="ExternalInput")
    t_iota = nc.dram_tensor("iota", [128, 128], MM_DT, kind="ExternalInput")
    t_ident = nc.dram_tensor("ident", [128, 128], f32, kind="ExternalInput")
    t_gidx = nc.dram_tensor("gidx", [128, total16], mybir.dt.int16,
                            kind="ExternalInput")
    t_dloc = nc.dram_tensor("dloc", [128, nch_tot], f32, kind="ExternalInput")
    t_diso = nc.dram_tensor("diso", [128, NTILE], f32, kind="ExternalInput")
    t_out = nc.dram_tensor("out", [RPAD, N_CLS], f32, kind="ExternalOutput")

    # internal DRAM
    g1slab = nc.dram_tensor("g1slab", [RPAD, H_GCN], MM_DT)
    g1tab = nc.dram_tensor("g1tab", [NPAD, H_GCN], MM_DT, addr_space="Shared")
    g2slab = nc.dram_tensor("g2slab", [RPAD, C_PAD], MM_DT)
    g2tab = nc.dram_tensor("g2tab", [NPAD, C_PAD], MM_DT, addr_space="Shared")

    with tile.TileContext(nc) as tc:
        with ExitStack() as octx:
            # constants, resident for the whole kernel
            cpool = octx.enter_context(tc.tile_pool(name="const", bufs=1))
            wmlp_sb = cpool.tile([128, 4 * H_MLP], f32)   # 4 K-chunks side by side
            for kc in range(4):
                nc.sync.dma_start(wmlp_sb[:, kc * H_MLP:(kc + 1) * H_MLP],
                                  t_wmlp[kc * 128:(kc + 1) * 128, :])
            w1_sb = cpool.tile([128, 2 * H_GCN], f32)
            for kc in range(2):
                nc.sync.dma_start(w1_sb[:, kc * H_GCN:(kc + 1) * H_GCN],
                                  t_w1[kc * 128:(kc + 1) * 128, :])
            w2_sb = cpool.tile([128, 2 * C_PAD], f32)
            for kc in range(2):
                nc.sync.dma_start(w2_sb[:, kc * C_PAD:(kc + 1) * C_PAD],
                                  t_w2[kc * 128:(kc + 1) * 128, :])
            bmlp_sb = cpool.tile([128, 2], f32)
            nc.sync.dma_start(bmlp_sb[:], t_bmlp[:])
            b1_sb = cpool.tile([128, H_GCN], f32)
            nc.sync.dma_start(b1_sb[:], t_b1[:])
            b2_sb = cpool.tile([128, C_PAD], f32)
            nc.sync.dma_start(b2_sb[:], t_b2[:])
            iota_sb = cpool.tile([128, 128], MM_DT)
            nc.sync.dma_start(iota_sb[:], t_iota[:])
            ident_sb = cpool.tile([128, 128], f32)
            nc.sync.dma_start(ident_sb[:], t_ident[:])
            diso_sb = cpool.tile([128, NTILE], f32)
            nc.sync.dma_start(diso_sb[:], t_diso[:])
            gidx_sb = cpool.tile([128, total16], mybir.dt.int16)
            for c0 in range(0, total16, 2048):
                c1 = min(c0 + 2048, total16)
                nc.sync.dma_start(gidx_sb[:, c0:c1], t_gidx[:, c0:c1])
            dloc_sb = cpool.tile([128, nch_tot], f32)
            for c0 in range(0, nch_tot, 1024):
                c1 = min(c0 + 1024, nch_tot)
                nc.sync.dma_start(dloc_sb[:, c0:c1], t_dloc[:, c0:c1])

            # ---------------- Phase A: h = relu(x@Wmlp+b); g1' = dis*(h@W1)
            with ExitStack() as ctx:
                xpool = ctx.enter_context(tc.tile_pool(name="xp", bufs=3))
                hpool = ctx.enter_context(tc.tile_pool(name="hp", bufs=3))
                opool = ctx.enter_context(tc.tile_pool(name="op", bufs=3))
                ps_h = ctx.enter_context(tc.tile_pool(name="psh", bufs=2, space="PSUM"))
                ps_g = ctx.enter_context(tc.tile_pool(name="psg", bufs=2, space="PSUM"))
                ps_t = ctx.enter_context(tc.tile_pool(name="pst", bufs=3, space="PSUM"))
                RT = 512
                n_it = (RPAD + RT - 1) // RT
                for it in range(n_it):
                    r0 = it * RT
                    rt = min(RT, RPAD - r0)
                    xt = xpool.tile([128, 4, RT], f32, tag="xt")
                    for kc in range(4):
                        nc.sync.dma_start(xt[:, kc, :rt],
                                          t_xT[kc * 128:(kc + 1) * 128, r0:r0 + rt])
                    ht = hpool.tile([128, 2, RT], f32, tag="ht")
                    for mh in range(2):
                        ph = ps_h.tile([128, RT], f32, tag="ph")
                        for kc in range(4):
                            nc.tensor.matmul(
                                ph[:, :rt],
                                wmlp_sb[:, kc * H_MLP + mh * 128:
                                        kc * H_MLP + (mh + 1) * 128],
                                xt[:, kc, :rt],
                                start=(kc == 0), stop=(kc == 3))
                        nc.scalar.activation(ht[:, mh, :rt], ph[:, :rt],
                                             mybir.ActivationFunctionType.Relu,
                                             bias=bmlp_sb[:, mh:mh + 1], scale=1.0)
                    g1t = hpool.tile([128, 2, RT], f32, tag="g1t")
                    for mh in range(2):
                        pg = ps_g.tile([128, RT], f32, tag="pg")
                        for kc in range(2):
                            nc.tensor.matmul(
                                pg[:, :rt],
                                w1_sb[:, kc * H_GCN + mh * 128:
                                      kc * H_GCN + (mh + 1) * 128],
                                ht[:, kc, :rt],
                                start=(kc == 0), stop=(kc == 1))
                        nc.vector.tensor_copy(g1t[:, mh, :rt], pg[:, :rt])
                    for rb in range(rt // 128):
                        tix = (r0 + rb * 128) // 128
                        pt = ps_t.tile([128, H_GCN], f32, tag="pt")
                        for mh in range(2):
                            nc.tensor.transpose(
                                pt[:, mh * 128:(mh + 1) * 128],
                                g1t[:, mh, rb * 128:(rb + 1) * 128],
                                ident_sb[:])
                        g1row = opool.tile([128, H_GCN], MM_DT, tag="g1row")
                        nc.vector.tensor_scalar_mul(g1row[:], pt[:],
                                                    diso_sb[:, tix:tix + 1])
                        nc.sync.dma_start(
                            g1slab[tix * 128:(tix + 1) * 128, :], g1row[:])

            # ---------------- Phase B: AllGather g1'
            if K_PHASES >= 2:
              nc.gpsimd.collective_compute(
                "AllGather", mybir.AluOpType.bypass,
                ins=[g1slab[:]], outs=[g1tab[:]],
                replica_groups=[list(range(NCORES))])

            # ---------------- Phase C: L1 aggregate + h1 + g2'
            if K_PHASES >= 3:
              with ExitStack() as ctx:
                mpool = ctx.enter_context(tc.tile_pool(name="msgs", bufs=6))
                ohpool = ctx.enter_context(tc.tile_pool(name="oh", bufs=4))
                hpool = ctx.enter_context(tc.tile_pool(name="h1p", bufs=3))
                ps_a = ctx.enter_context(tc.tile_pool(name="psa", bufs=2, space="PSUM"))
                ps_t = ctx.enter_context(tc.tile_pool(name="pst2", bufs=3, space="PSUM"))
                ps_2 = ctx.enter_context(tc.tile_pool(name="ps2", bufs=2, space="PSUM"))
                gsmax = int(gs128.max())
                for t in range(NTILE):
                    goff_t = int(G[:t].sum()) if t else 0
                    pa = ps_a.tile([128, H_GCN], f32, tag="pa")
                    chunk_base = goff_t // 128
                    ci = 0
                    n_t = int(nch[t])
                    for s in range(NSEG):
                        gsz = int(G[t, s])
                        if gsz == 0:
                            continue
                        off16 = (goff_t + int(G[t, :s].sum())) // 16
                        msgs = mpool.tile([128, gsmax, H_GCN], MM_DT, tag="m1")
                        nc.gpsimd.dma_gather(
                            out_ap=msgs[:, :gsz // 128, :],
                            in_ap=g1tab[s * SEGR:(s + 1) * SEGR, :],
                            idxs_ap=gidx_sb[:, off16:off16 + gsz // 16],
                            num_idxs=gsz, num_idxs_reg=gsz, elem_size=H_GCN,
                            single_packet=False)
                        for k in range(gsz // 128):
                            oh = ohpool.tile([128, 128], MM_DT, tag="oh")
                            nc.vector.tensor_scalar(
                                oh[:], iota_sb[:],
                                dloc_sb[:, chunk_base + ci:chunk_base + ci + 1],
                                None, mybir.AluOpType.is_equal)
                            nc.tensor.matmul(pa[:], oh[:], msgs[:, k, :],
                                             start=(ci == 0), stop=(ci == n_t - 1))
                            ci += 1
                    h1 = hpool.tile([128, H_GCN], f32, tag="h1")
                    if b1_nonzero:
                        nc.vector.tensor_scalar_mul(h1[:], pa[:],
                                                    diso_sb[:, t:t + 1])
                        nc.vector.tensor_add(h1[:], h1[:], b1_sb[:])
                        nc.scalar.activation(h1[:], h1[:],
                                             mybir.ActivationFunctionType.Relu,
                                             bias=0.0, scale=1.0)
                    else:
                        nc.scalar.activation(h1[:], pa[:],
                                             mybir.ActivationFunctionType.Relu,
                                             bias=0.0, scale=diso_sb[:, t:t + 1])
                    # g2 = h1 @ W2, scaled by dis
                    pt = ps_t.tile([128, 2, 128], f32, tag="ptc")
                    h1t = hpool.tile([128, 2, 128], f32, tag="h1t")
                    p2 = ps_2.tile([128, C_PAD], f32, tag="p2")
                    for kk in range(2):
                        nc.tensor.transpose(pt[:, kk, :],
                                            h1[:, kk * 128:(kk + 1) * 128],
                                            ident_sb[:])
                        nc.vector.tensor_copy(h1t[:, kk, :], pt[:, kk, :])
                    for kk in range(2):
                        nc.tensor.matmul(p2[:], h1t[:, kk, :],
                                         w2_sb[:, kk * C_PAD:(kk + 1) * C_PAD],
                                         start=(kk == 0), stop=(kk == 1))
                    g2row = hpool.tile([128, C_PAD], MM_DT, tag="g2row")
                    nc.vector.tensor_scalar_mul(g2row[:], p2[:],
                                                diso_sb[:, t:t + 1])
                    nc.sync.dma_start(g2slab[t * 128:(t + 1) * 128, :], g2row[:])

            # ---------------- Phase D: AllGather g2'
            if K_PHASES >= 4:
              nc.gpsimd.collective_compute(
                "AllGather", mybir.AluOpType.bypass,
                ins=[g2slab[:]], outs=[g2tab[:]],
                replica_groups=[list(range(NCORES))])

            # ---------------- Phase E: L2 aggregate -> out
            if K_PHASES >= 5:
              with ExitStack() as ctx:
                mpool = ctx.enter_context(tc.tile_pool(name="msgs2", bufs=6))
                ohpool = ctx.enter_context(tc.tile_pool(name="oh2", bufs=4))
                hpool = ctx.enter_context(tc.tile_pool(name="outp", bufs=3))
                ps_a = ctx.enter_context(tc.tile_pool(name="psa2", bufs=2, space="PSUM"))
                gsmax = int(gs128.max())
                for t in range(NTILE):
                    goff_t = int(G[:t].sum()) if t else 0
                    pa = ps_a.tile([128, C_PAD], f32, tag="pa2")
                    chunk_base = goff_t // 128
                    ci = 0
                    n_t = int(nch[t])
                    for s in range(NSEG):
                        gsz = int(G[t, s])
                        if gsz == 0:
                            continue
                        off16 = (goff_t + int(G[t, :s].sum())) // 16
                        msgs = mpool.tile([128, gsmax, C_PAD], MM_DT, tag="m2")
                        nc.gpsimd.dma_gather(
                            out_ap=msgs[:, :gsz // 128, :],
                            in_ap=g2tab[s * SEGR:(s + 1) * SEGR, :],
                            idxs_ap=gidx_sb[:, off16:off16 + gsz // 16],
                            num_idxs=gsz, num_idxs_reg=gsz, elem_size=C_PAD,
                            single_packet=False)
                        for k in range(gsz // 128):
                            oh = ohpool.tile([128, 128], MM_DT, tag="oh2")
                            nc.vector.tensor_scalar(
                                oh[:], iota_sb[:],
                                dloc_sb[:, chunk_base + ci:chunk_base + ci + 1],
                                None, mybir.AluOpType.is_equal)
                            nc.tensor.matmul(pa[:], oh[:], msgs[:, k, :],
                                             start=(ci == 0), stop=(ci == n_t - 1))
                            ci += 1
                    ot = hpool.tile([128, C_PAD], f32, tag="ot")
                    nc.vector.tensor_scalar_mul(ot[:], pa[:], diso_sb[:, t:t + 1])
                    nc.vector.tensor_add(ot[:], ot[:], b2_sb[:])
                    nc.sync.dma_start(t_out[t * 128:(t + 1) * 128, :],
                                      ot[:, :N_CLS])

    nc.compile()
    _split_multi_waits(nc)
    return nc


def _split_multi_waits(nc, max_waits=1):
    """walrus CoreV3 rejects >max_waits sem waits on one instruction; split
    extras onto preceding NOPs on the same engine."""
    n = 0
    for fn in nc.m.functions:
        for bb in fn.blocks:
            insts = bb.instructions
            i = 0
            while i < len(insts):
                inst = insts[i]
                si = inst.sync_info
                if si is not None and si.on_wait and len(si.on_wait) > max_waits:
                    waits = list(si.on_wait)
                    keep = waits[-max_waits:]
                    extra = waits[:-max_waits]
                    new_insts = []
                    for cs in range(0, len(extra), max_waits):
                        nop = mybir.InstNoOp(
                            name=f"I-waitsplit-{id(inst)}-{cs}-{n}",
                            sync_info=mybir.SyncInfo(
                                on_wait=extra[cs:cs + max_waits], on_update=[]),
                            bass_nofuse=True,
                            engine=inst.engine)
                        new_insts.append(nop)
                        n += 1
                    si.on_wait = keep
                    for j, nop in enumerate(new_insts):
                        insts.insert(i + j, nop)
                    i += len(new_insts)
                i += 1
    return n


def prepare(x, edge_index, W_mlp, b_mlp, W1, b1, W2, b2):
    x = np.asarray(x, dtype=np.float32)
    W_mlp_ = np.asarray(W_mlp, dtype=np.float32)
    b_mlp_ = np.asarray(b_mlp, dtype=np.float32)
    W1_ = np.asarray(W1, dtype=np.float32)
    b1_ = np.asarray(b1, dtype=np.float32)
    W2_ = np.asarray(W2, dtype=np.float32)
    b2_ = np.asarray(b2, dtype=np.float32)

    G, Goff, total, gidx, dloc, diso = _preprocess(edge_index)
    nc = _build_program(G, total, b1_nonzero=bool(np.abs(b1_).max() > 0))

    W2p = np.zeros((H_GCN, C_PAD), np.float32)
    W2p[:, :N_CLS] = W2_
    b2p = np.zeros(C_PAD, np.float32)
    b2p[:N_CLS] = b2_
    bmlp_pk = np.ascontiguousarray(b_mlp_.reshape(2, 128).T)
    b1bc = np.tile(b1_[None, :], (128, 1)).astype(np.float32)
    b2bc = np.tile(b2p[None, :], (128, 1)).astype(np.float32)
    iota = np.tile(np.arange(128, dtype=np.float32)[None, :], (128, 1)).astype(MM_NP)
    ident = np.eye(128, dtype=np.float32)

    in_maps = []
    for c in range(NCORES):
        xs = np.zeros((RPAD, N_FEAT), np.float32)
        xs[:RPC] = x[c * RPC:(c + 1) * RPC]
        in_maps.append({
            "xT": np.ascontiguousarray(xs.T),
            "wmlp": W_mlp_, "w1": W1_, "w2": W2p,
            "bmlp": bmlp_pk, "b1bc": b1bc, "b2bc": b2bc,
            "iota": iota, "ident": ident,
            "gidx": gidx[c], "dloc": dloc[c], "diso": diso[c],
        })

    return nc, in_maps


def kernel(**inputs):
    nc, in_maps = prepare(**inputs)
    res = run_bass_kernel_spmd(nc, in_maps, list(range(NCORES)))
    global last_results
    last_results = res
    out = np.concatenate(
        [res.results[c]["out"][:RPC] for c in range(NCORES)], axis=0)
    return out.astype(np.float32)


last_results = None


if __name__ == "__main__":
    import reference
    from np_ref import np_reference
    inputs = {k: np.asarray(v) for k, v in reference.setup_inputs().items()}
    got = kernel(**inputs)
    exp = np_reference(**inputs)
    denom = np.abs(exp).max()
    err = np.abs(got - exp).max()
    print(f"abs err {err}  rel err {err / denom}  scale {denom}")

